# revision 6
# baseline (speedup 1.0000x reference)
"""Trainium2 Bass kernel for the Attractor recurrence.

Problem: hs_{t+1} = l2norm(leaky_relu(0.5*hs_t + h_t @ M)), 16 steps,
B=8, D=8192, M is 8192x8192 f32.

Math restructuring used here:
  * leaky_relu is positively homogeneous and l2norm is scale invariant, so
    the per-step normalization cancels out of the recurrence entirely.  We
    iterate the unnormalized map  w -> lrelu(0.5*w + w @ M)  with a fixed
    2^-12 rescale per step folded in to keep magnitudes bounded, and
    normalize once on the host at the end.
  * the decay term 0.5*w is linear, so it is baked into the matrix:
    M'' = lam * (M + 0.5*I).  The device loop is then purely
    w -> lrelu(w @ M'').  Step 1 of the reference uses h=x with hs=0 (no
    decay), so the baked decay is subtracted back out on step 1 only.
  * M'' is cast to bf16 (verified 1e-4 max rel err end-to-end), which lets
    each core's 8192x1024 column shard stay resident in SBUF -- M is read
    from HBM exactly once instead of once per step.

Sharding: M'' column-sharded across 8 cores.  Each step, core r computes
its [8, 1024] slice of w @ M'', applies leaky-relu, transposes to [1024, 8]
via the PE, and AllGathers the bf16 shards so every core has the full
transposed state [8192, 8] (the exact stationary-operand layout the next
matmul needs).  The final step skips the gather; each core writes its f32
column shard and the host concatenates + normalizes.
"""

import numpy as np
import ml_dtypes

B = 8          # batch
D = 8192       # feature dim
NCORES = 8
DK = D // NCORES       # 1024 columns per core
KT = D // 128          # 64 K-tiles of 128
TAU = 16
SLOPE = 0.01
LAM = float(2.0 ** -12)

_BF16 = ml_dtypes.bfloat16

_cached = {}


def _build_program(tau=TAU):
    """Build the SPMD Bass/Tile program (same program runs on all 8 cores)."""
    import concourse.bass as bass
    import concourse.mybir as mybir
    import concourse.tile as tile
    from concourse import bacc

    fp32 = mybir.dt.float32
    bf16 = mybir.dt.bfloat16
    ALU = mybir.AluOpType

    nc = bacc.Bacc(
        "TRN2",
        target_bir_lowering=False,
        debug=False,
        num_devices=NCORES,
    )

    # Kernel I/O (per-core data differs, program is shared).
    m_dram = nc.dram_tensor("m", [D, DK], bf16, kind="ExternalInput")
    xt_dram = nc.dram_tensor("xt", [128, KT * B], bf16, kind="ExternalInput")
    xsh_dram = nc.dram_tensor("xsh", [B, DK], bf16, kind="ExternalInput")
    ident_dram = nc.dram_tensor("ident", [B, B], bf16, kind="ExternalInput")
    out_dram = nc.dram_tensor("out", [B, DK], fp32, kind="ExternalOutput")

    with tile.TileContext(nc, num_cores=NCORES) as tc:
        with (
            tc.tile_pool(name="mpool", bufs=1) as mpool,
            tc.tile_pool(name="consts", bufs=1) as consts,
            tc.tile_pool(name="state", bufs=2) as state,
            tc.tile_pool(name="qpool", bufs=2) as qpool,
            tc.tile_pool(name="tvec", bufs=2) as tvec,
            tc.tile_pool(name="fin", bufs=1) as fin,
            tc.tile_pool(name="mmps", bufs=2, space="PSUM") as mmps,
            tc.tile_pool(name="trps", bufs=2, space="PSUM") as trps,
            tc.tile_pool(name="dram", bufs=2, space="DRAM") as dram,
        ):
            # --- resident M'' shard: [128, KT*1024] bf16, K-tile ki at
            # columns [ki*1024, (ki+1)*1024) ---
            m_sb = mpool.tile([128, KT * DK], bf16)
            m_view = m_dram.ap().rearrange("(kt p) c -> kt p c", p=128)
            NGRP = 16
            KT_PER_GRP = KT // NGRP
            for g in range(NGRP):
                src = m_view[g * KT_PER_GRP : (g + 1) * KT_PER_GRP].rearrange(
                    "kt p c -> p kt c"
                )
                dst = m_sb[:, g * KT_PER_GRP * DK : (g + 1) * KT_PER_GRP * DK]
                nc.sync.dma_start(out=dst.rearrange("p (kt c) -> p kt c", c=DK),
                                  in_=src)

            # --- constants ---
            xt_sb = consts.tile([128, KT * B], bf16)
            nc.sync.dma_start(out=xt_sb[:], in_=xt_dram.ap())
            xsh_sb = consts.tile([B, DK], bf16)
            nc.sync.dma_start(out=xsh_sb[:], in_=xsh_dram.ap())
            ident_sb = consts.tile([B, B], bf16)
            nc.sync.dma_start(out=ident_sb[:], in_=ident_dram.ap())

            cur_vT = xt_sb  # iteration-1 stationary operand = bf16(x)^T

            for t in range(tau):
                last = t == tau - 1

                # --- w @ M'' : 128 accumulating matmuls into [8, 1024] psum
                mm_ps = mmps.tile([B, DK], fp32)
                for ki in range(KT):
                    lhsT = cur_vT[:, ki * B : (ki + 1) * B]
                    for half in range(2):
                        nc.tensor.matmul(
                            mm_ps[:, half * 512 : (half + 1) * 512],
                            lhsT,
                            m_sb[:, ki * DK + half * 512 : ki * DK + half * 512 + 512],
                            start=(ki == 0),
                            stop=(ki == KT - 1),
                        )

                if last:
                    # f32 leaky-relu on the shard, write output; host
                    # normalizes (scale drops out).
                    a_f = fin.tile([B, DK], fp32)
                    nc.vector.tensor_scalar_mul(a_f[:], mm_ps[:], SLOPE)
                    o_f = fin.tile([B, DK], fp32)
                    nc.vector.tensor_tensor(
                        out=o_f[:], in0=mm_ps[:], in1=a_f[:], op=ALU.max
                    )
                    nc.sync.dma_start(out=out_dram.ap(), in_=o_f[:])
                    continue

                # --- q = lam*pre in bf16 (step 1: subtract the baked decay
                # since the reference's first step has hs=0) ---
                q_sb = qpool.tile([B, DK], bf16)
                if t == 0:
                    nc.vector.scalar_tensor_tensor(
                        out=q_sb[:],
                        in0=xsh_sb[:],
                        scalar=-0.5 * LAM,
                        in1=mm_ps[:],
                        op0=ALU.mult,
                        op1=ALU.add,
                    )
                else:
                    nc.vector.tensor_copy(out=q_sb[:], in_=mm_ps[:])

                # --- transpose q to [1024, 8] == [128, 64] via PE ---
                tr_ps = trps.tile([128, B * B], bf16)
                for m in range(8):
                    nc.tensor.transpose(
                        tr_ps[:, m * B : (m + 1) * B],
                        q_sb[:, m * 128 : (m + 1) * 128],
                        ident_sb[:],
                    )

                # --- leaky-relu in T orientation: w = max(q, 0.01*q) ---
                a_T = tvec.tile([128, B * B], bf16)
                nc.vector.tensor_scalar_mul(a_T[:], tr_ps[:], SLOPE)
                w_T = tvec.tile([128, B * B], bf16)
                nc.vector.tensor_tensor(
                    out=w_T[:], in0=tr_ps[:], in1=a_T[:], op=ALU.max
                )

                # --- AllGather the bf16 shard; result is the full
                # transposed state in exactly the layout the next matmul
                # reads ---
                ag_in = dram.tile([128 * B * B], bf16)
                ag_out = dram.tile([NCORES * 128 * B * B], bf16)
                nc.sync.dma_start(
                    out=ag_in.rearrange("(p c) -> p c", p=128), in_=w_T[:]
                )
                nc.gpsimd.collective_compute(
                    "AllGather",
                    ALU.bypass,
                    replica_groups=[list(range(NCORES))],
                    ins=[ag_in[:]],
                    outs=[ag_out[:]],
                )
                vT = state.tile([128, KT * B], bf16)
                nc.sync.dma_start(
                    out=vT.rearrange("p (r c) -> p r c", r=NCORES),
                    in_=ag_out.rearrange("(r p c) -> p r c", p=128, c=B * B),
                )
                cur_vT = vT

    nc.finalize()
    return nc


def _get_program(tau=TAU):
    if tau not in _cached:
        _cached[tau] = _build_program(tau)
    return _cached[tau]


def _prep_inputs(x, M):
    """Host-side shard prep. Returns list of 8 per-core input dicts."""
    lam = np.float32(LAM)
    # M'' = lam * (M + 0.5*I), bf16, column-sharded
    ident_scaled = np.zeros((), dtype=np.float32)  # placeholder
    xt = (
        x.reshape(B, KT, 128)
        .transpose(2, 1, 0)  # [128, KT, B]
        .reshape(128, KT * B)
        .astype(_BF16)
    )
    ident = np.eye(B, dtype=np.float32).astype(_BF16)
    in_maps = []
    for r in range(NCORES):
        cols = slice(r * DK, (r + 1) * DK)
        m_shard = M[:, cols] * lam
        # add baked decay on the diagonal block rows
        m_shard = np.ascontiguousarray(m_shard)
        idx = np.arange(DK)
        m_shard[r * DK + idx, idx] += np.float32(0.5) * lam
        in_maps.append(
            {
                "m": m_shard.astype(_BF16),
                "xt": xt,
                "xsh": np.ascontiguousarray(x[:, cols]).astype(_BF16),
                "ident": ident,
            }
        )
    return in_maps


def kernel(x, M, hs):
    """Full-input entry point: shards internally across 8 NeuronCores."""
    from concourse.bass_utils import run_bass_kernel_spmd

    x = np.asarray(x, dtype=np.float32)
    M = np.asarray(M, dtype=np.float32)
    nc = _get_program()
    in_maps = _prep_inputs(x, M)
    res = run_bass_kernel_spmd(nc, in_maps, core_ids=list(range(NCORES)))
    shards = [res.results[r]["out"] for r in range(NCORES)]
    v = np.concatenate(shards, axis=1)  # [8, 8192] f32, unnormalized act_16
    # Normalize in f64 WITHOUT the reference's 1e-12 clamp: our v carries an
    # arbitrary per-row scale (collapsed all-negative rows shrink ~100x per
    # step), so the clamp must scale with it; the reference's clamp never
    # fires for its own normalized state.
    v64 = v.astype(np.float64)
    nrm = np.sqrt((v64 ** 2).sum(axis=1, keepdims=True))
    return (v64 / nrm).astype(np.float32)


# revision 9
# speedup vs baseline: 1.2400x; 1.2400x over previous
"""Trainium2 Bass kernel for the Attractor recurrence.

Problem: hs_{t+1} = l2norm(leaky_relu(0.5*hs_t + h_t @ M)), 16 steps,
B=8, D=8192, M is 8192x8192 f32.

Math restructuring used here:
  * leaky_relu is positively homogeneous and l2norm is scale invariant, so
    the per-step normalization cancels out of the recurrence entirely.  We
    iterate the unnormalized map  w -> lrelu(0.5*w + w @ M)  with a fixed
    2^-12 rescale per step folded in to keep magnitudes bounded, and
    normalize once on the host at the end.
  * the decay term 0.5*w is linear, so it is baked into the matrix:
    M'' = lam * (M + 0.5*I).  The device loop is then purely
    w -> lrelu(w @ M'').  Step 1 of the reference uses h=x with hs=0 (no
    decay), so the baked decay is subtracted back out on step 1 only.
  * M'' is cast to bf16 (verified ~1e-4 max rel err end-to-end), which lets
    each core's 8192x1024 column shard stay resident in SBUF -- M is read
    from HBM exactly once instead of once per step.

Sharding: M'' column-sharded across 8 cores.  Each step, core r computes
its [8, 1024] slice of w @ M'', applies leaky-relu, transposes to [1024, 8]
via the PE, and AllGathers the bf16 shards so every core has the full
transposed state [8192, 8] (the exact stationary-operand layout the next
matmul needs).  The final step skips the gather; each core writes its f32
column shard and the host concatenates + normalizes.

Pipelining (v2): each iteration's output is split into two 512-column
halves with separate AllGathers.  Contraction K-tiles are grouped into
A (ki%8<4, covered by AG#1 of the previous step) and B (covered by AG#2).
MM emission order A0,A1a,B0,A1b,B1 makes half-0 finish ~60% into the MM
stream, so AG#1 is in flight while B-half matmuls still run and lands just
in time for the next iteration's A matmuls; AG#2's latency hides under the
next iteration's A work.  The PE therefore never idles long enough to lose
its HAM clock boost.  A dummy warm-up AllGather absorbs the expensive
first-collective staging during the (overlapped) M load.
"""

import numpy as np
import ml_dtypes

B = 8          # batch
D = 8192       # feature dim
NCORES = 8
DK = D // NCORES       # 1024 columns per core
KT = D // 128          # 64 K-tiles of 128
TAU = 16
SLOPE = 0.01
LAM = float(2.0 ** -12)

_BF16 = ml_dtypes.bfloat16

_cached = {}


def _build_program(tau=TAU):
    """Build the SPMD Bass/Tile program (same program runs on all 8 cores)."""
    import concourse.bass as bass
    import concourse.mybir as mybir
    import concourse.tile as tile
    from concourse import bacc

    fp32 = mybir.dt.float32
    bf16 = mybir.dt.bfloat16
    ALU = mybir.AluOpType
    RG = [list(range(NCORES))]

    nc = bacc.Bacc(
        "TRN2",
        target_bir_lowering=False,
        debug=False,
        num_devices=NCORES,
    )

    # Kernel I/O (per-core data differs, program is shared).
    m_dram = nc.dram_tensor("m", [D, DK], bf16, kind="ExternalInput")
    xt_dram = nc.dram_tensor("xt", [128, KT * B], bf16, kind="ExternalInput")
    xsh_dram = nc.dram_tensor("xsh", [B, DK], bf16, kind="ExternalInput")
    ident_dram = nc.dram_tensor("ident", [B, B], bf16, kind="ExternalInput")
    out_dram = nc.dram_tensor("out", [B, DK], fp32, kind="ExternalOutput")

    # K-tile contraction groups: A covered by AG#1, B by AG#2.
    A_KI = [ki for ki in range(KT) if ki % 8 < 4]
    B_KI = [ki for ki in range(KT) if ki % 8 >= 4]

    with tile.TileContext(nc, num_cores=NCORES) as tc:
        with (
            tc.tile_pool(name="mpool", bufs=1) as mpool,
            tc.tile_pool(name="consts", bufs=1) as consts,
            tc.tile_pool(name="state", bufs=2) as state,
            tc.tile_pool(name="qpool", bufs=3) as qpool,
            tc.tile_pool(name="tvec", bufs=3) as tvec,
            tc.tile_pool(name="fin", bufs=1) as fin,
            tc.tile_pool(name="mmps", bufs=3, space="PSUM") as mmps,
            tc.tile_pool(name="trps", bufs=3, space="PSUM") as trps,
            tc.tile_pool(name="dram", bufs=3, space="DRAM") as dram,
        ):
            # --- dummy warm-up AllGather: absorbs first-collective staging
            # cost (~50us) while the M shard streams in ---
            ident_sb = consts.tile([B, B], bf16)
            nc.sync.dma_start(out=ident_sb[:], in_=ident_dram.ap())
            warm_in = dram.tile([64], bf16, tag="warm_in")
            warm_out = dram.tile([64 * NCORES], bf16, tag="warm_out")
            nc.sync.dma_start(
                out=warm_in.rearrange("(p c) -> p c", p=8), in_=ident_sb[:]
            )
            nc.gpsimd.collective_compute(
                "AllGather", ALU.bypass, replica_groups=RG,
                ins=[warm_in[:]], outs=[warm_out[:]],
            )

            # --- resident M'' shard: 16 tiles of 4 K-tiles each so matmuls
            # can start as soon as their group has landed.  A-groups (even)
            # are loaded first to match iteration-1's MM order. ---
            m_view = m_dram.ap().rearrange("(kt p) c -> kt p c", p=128)
            m_tiles = {}
            for g in [x for x in range(16) if x % 2 == 0] + [
                x for x in range(16) if x % 2 == 1
            ]:
                mt = mpool.tile([128, 4 * DK], bf16, tag=f"m{g}")
                src = m_view[g * 4 : (g + 1) * 4].rearrange("kt p c -> p kt c")
                nc.sync.dma_start(
                    out=mt.rearrange("p (kt c) -> p kt c", c=DK), in_=src
                )
                m_tiles[g] = mt

            def m_ap(ki, col0, ncol):
                """AP of M'' K-tile ki, columns [col0, col0+ncol)."""
                g, kk = divmod(ki, 4)
                return m_tiles[g][:, kk * DK + col0 : kk * DK + col0 + ncol]

            # --- constants ---
            xt_sb = consts.tile([128, KT * B], bf16)
            nc.sync.dma_start(out=xt_sb[:], in_=xt_dram.ap())
            xsh_sb = consts.tile([B, DK], bf16)
            nc.sync.dma_start(out=xsh_sb[:], in_=xsh_dram.ap())

            cur_vT = xt_sb  # iteration-1 stationary operand = bf16(x)^T

            for t in range(tau):
                last = t == tau - 1

                ps = [
                    mmps.tile([B, 512], fp32, tag="ps", name=f"ps{t}_{h}")
                    for h in range(2)
                ]
                nxt_vT = None if last else state.tile([128, KT * B], bf16)

                def mm_block(kis, half, start, stop):
                    for i, ki in enumerate(kis):
                        nc.tensor.matmul(
                            ps[half][:],
                            cur_vT[:, ki * B : (ki + 1) * B],
                            m_ap(ki, half * 512, 512),
                            start=(start and i == 0),
                            stop=(stop and i == len(kis) - 1),
                        )

                def half_tail(half):
                    """cast -> transpose -> lrelu -> DMA out -> AllGather ->
                    DMA into the next state tile (emitted per half)."""
                    q = qpool.tile([B, 512], bf16, tag="q")
                    if t == 0:
                        # subtract the baked decay (reference step 1 has hs=0)
                        nc.vector.scalar_tensor_tensor(
                            out=q[:],
                            in0=xsh_sb[:, half * 512 : half * 512 + 512],
                            scalar=-0.5 * LAM,
                            in1=ps[half][:],
                            op0=ALU.mult,
                            op1=ALU.add,
                        )
                    else:
                        nc.vector.tensor_copy(out=q[:], in_=ps[half][:])
                    tr = trps.tile([128, 4 * B], bf16, tag="tr")
                    for m in range(4):
                        nc.tensor.transpose(
                            tr[:, m * B : (m + 1) * B],
                            q[:, m * 128 : (m + 1) * 128],
                            ident_sb[:],
                        )
                    a_T = tvec.tile([128, 4 * B], bf16, tag="aT")
                    nc.vector.tensor_scalar_mul(a_T[:], tr[:], SLOPE)
                    w_T = tvec.tile([128, 4 * B], bf16, tag="wT")
                    nc.vector.tensor_tensor(
                        out=w_T[:], in0=tr[:], in1=a_T[:], op=ALU.max
                    )
                    ag_in = dram.tile([128 * 4 * B], bf16, tag="ag_in")
                    ag_out = dram.tile([NCORES * 128 * 4 * B], bf16, tag="ag_out")
                    nc.sync.dma_start(
                        out=ag_in.rearrange("(p c) -> p c", p=128), in_=w_T[:]
                    )
                    nc.gpsimd.collective_compute(
                        "AllGather", ALU.bypass, replica_groups=RG,
                        ins=[ag_in[:]], outs=[ag_out[:]],
                    )
                    # gathered rank blocks -> interleaved state columns:
                    # rank r half h lands at vT[:, r*64 + 32h : r*64 + 32h+32]
                    dst = nxt_vT[:].rearrange("p (r c) -> p r c", c=8 * B)[
                        :, :, half * 4 * B : (half + 1) * 4 * B
                    ]
                    nc.sync.dma_start(
                        out=dst,
                        in_=ag_out.rearrange("(r p c) -> p r c", p=128, c=4 * B),
                    )

                if last:
                    # f32 leaky-relu on the shard, write output; host
                    # normalizes (scale drops out).
                    mm_block(A_KI, 0, True, False)
                    mm_block(A_KI, 1, True, False)
                    mm_block(B_KI, 0, False, True)
                    mm_block(B_KI, 1, False, True)
                    o_f = fin.tile([B, DK], fp32)
                    for half in range(2):
                        a_f = fin.tile([B, 512], fp32, tag="af")
                        nc.vector.tensor_scalar_mul(a_f[:], ps[half][:], SLOPE)
                        nc.vector.tensor_tensor(
                            out=o_f[:, half * 512 : half * 512 + 512],
                            in0=ps[half][:],
                            in1=a_f[:],
                            op=ALU.max,
                        )
                    nc.sync.dma_start(out=out_dram.ap(), in_=o_f[:])
                    continue

                # MM emission order: A0, A1a, B0, [h0 tail], A1b, B1, [h1 tail]
                mm_block(A_KI, 0, True, False)
                mm_block(A_KI[:16], 1, True, False)
                mm_block(B_KI, 0, False, True)
                half_tail(0)
                mm_block(A_KI[16:], 1, False, False)
                mm_block(B_KI, 1, False, True)
                half_tail(1)

                cur_vT = nxt_vT

    nc.finalize()
    return nc


def _get_program(tau=TAU):
    if tau not in _cached:
        _cached[tau] = _build_program(tau)
    return _cached[tau]


def _prep_inputs(x, M):
    """Host-side shard prep. Returns list of 8 per-core input dicts."""
    lam = np.float32(LAM)
    xt = (
        x.reshape(B, KT, 128)
        .transpose(2, 1, 0)  # [128, KT, B]
        .reshape(128, KT * B)
        .astype(_BF16)
    )
    ident = np.eye(B, dtype=np.float32).astype(_BF16)
    in_maps = []
    idx = np.arange(DK)
    for r in range(NCORES):
        cols = slice(r * DK, (r + 1) * DK)
        m_shard = M[:, cols] * lam
        m_shard[r * DK + idx, idx] += np.float32(0.5) * lam
        in_maps.append(
            {
                "m": m_shard.astype(_BF16),
                "xt": xt,
                "xsh": np.ascontiguousarray(x[:, cols]).astype(_BF16),
                "ident": ident,
            }
        )
    return in_maps


def kernel(x, M, hs):
    """Full-input entry point: shards internally across 8 NeuronCores."""
    from concourse.bass_utils import run_bass_kernel_spmd

    x = np.asarray(x, dtype=np.float32)
    M = np.asarray(M, dtype=np.float32)
    nc = _get_program()
    in_maps = _prep_inputs(x, M)
    res = run_bass_kernel_spmd(nc, in_maps, core_ids=list(range(NCORES)))
    shards = [res.results[r]["out"] for r in range(NCORES)]
    v = np.concatenate(shards, axis=1)  # [8, 8192] f32, unnormalized act_16
    # Normalize in f64 WITHOUT the reference's 1e-12 clamp: our v carries an
    # arbitrary per-row scale (collapsed all-negative rows shrink ~100x per
    # step), so the clamp must scale with it; the reference's clamp never
    # fires for its own normalized state.
    v64 = v.astype(np.float64)
    nrm = np.sqrt((v64 ** 2).sum(axis=1, keepdims=True))
    return (v64 / nrm).astype(np.float32)


# revision 13
# speedup vs baseline: 1.2640x; 1.0193x over previous
"""Trainium2 Bass kernel for the Attractor recurrence.

Problem: hs_{t+1} = l2norm(leaky_relu(0.5*hs_t + h_t @ M)), 16 steps,
B=8, D=8192, M is 8192x8192 f32.

Math restructuring used here:
  * leaky_relu is positively homogeneous and l2norm is scale invariant, so
    the per-step normalization cancels out of the recurrence entirely.  We
    iterate the unnormalized map  w -> lrelu(0.5*w + w @ M)  with a fixed
    2^-12 rescale per step folded in to keep magnitudes bounded, and
    normalize once on the host at the end.
  * the decay term 0.5*w is linear, so it is baked into the matrix:
    M'' = lam * (M + 0.5*I).  The device loop is then purely
    w -> lrelu(w @ M'').  Step 1 of the reference uses h=x with hs=0 (no
    decay), so the baked decay is subtracted back out on step 1 only.
  * M'' is cast to bf16 (verified ~1e-4 max rel err end-to-end), which lets
    each core's 8192x1024 column shard stay resident in SBUF -- M is read
    from HBM exactly once instead of once per step.

Sharding: M'' column-sharded across 8 cores.  Each step, core r computes
its [8, 1024] slice of w @ M'', applies leaky-relu, transposes to [1024, 8]
via the PE, and AllGathers the bf16 shards so every core has the full
transposed state [8192, 8] (the exact stationary-operand layout the next
matmul needs).  The final step skips the gather; each core writes its f32
column shard and the host concatenates + normalizes.

Pipelining (v2): each iteration's output is split into two 512-column
halves with separate AllGathers.  Contraction K-tiles are grouped into
A (ki%8<4, covered by AG#1 of the previous step) and B (covered by AG#2).
MM emission order A0,A1a,B0,A1b,B1 makes half-0 finish ~60% into the MM
stream, so AG#1 is in flight while B-half matmuls still run and lands just
in time for the next iteration's A matmuls; AG#2's latency hides under the
next iteration's A work.  The PE therefore never idles long enough to lose
its HAM clock boost.  A dummy warm-up AllGather absorbs the expensive
first-collective staging during the (overlapped) M load.
"""

import numpy as np
import ml_dtypes

B = 8          # batch
D = 8192       # feature dim
NCORES = 8
DK = D // NCORES       # 1024 columns per core
KT = D // 128          # 64 K-tiles of 128
TAU = 16
SLOPE = 0.01
LAM = float(2.0 ** -12)

_BF16 = ml_dtypes.bfloat16

_cached = {}


def _build_program(tau=TAU):
    """Build the SPMD Bass/Tile program (same program runs on all 8 cores)."""
    import concourse.bass as bass
    import concourse.mybir as mybir
    import concourse.tile as tile
    from concourse import bacc

    fp32 = mybir.dt.float32
    bf16 = mybir.dt.bfloat16
    ALU = mybir.AluOpType
    RG = [list(range(NCORES))]

    nc = bacc.Bacc(
        "TRN2",
        target_bir_lowering=False,
        debug=False,
        num_devices=NCORES,
    )

    # Kernel I/O (per-core data differs, program is shared).
    m_dram = nc.dram_tensor("m", [D, DK], bf16, kind="ExternalInput")
    xt_dram = nc.dram_tensor("xt", [128, KT * B], bf16, kind="ExternalInput")
    xsh_dram = nc.dram_tensor("xsh", [B, DK], bf16, kind="ExternalInput")
    ident_dram = nc.dram_tensor("ident", [B, B], bf16, kind="ExternalInput")
    out_dram = nc.dram_tensor("out", [B, DK], fp32, kind="ExternalOutput")

    # K-tile contraction groups: A covered by AG#1, B by AG#2.
    A_KI = [ki for ki in range(KT) if ki % 8 < 4]
    B_KI = [ki for ki in range(KT) if ki % 8 >= 4]

    with tile.TileContext(nc, num_cores=NCORES) as tc:
        with (
            tc.tile_pool(name="mpool", bufs=1) as mpool,
            tc.tile_pool(name="consts", bufs=1) as consts,
            tc.tile_pool(name="state", bufs=2) as state,
            tc.tile_pool(name="qpool", bufs=3) as qpool,
            tc.tile_pool(name="tvec", bufs=3) as tvec,
            tc.tile_pool(name="fin", bufs=1) as fin,
            tc.tile_pool(name="mmps", bufs=3, space="PSUM") as mmps,
            tc.tile_pool(name="trps", bufs=3, space="PSUM") as trps,
            tc.tile_pool(name="dram", bufs=3, space="DRAM") as dram,
        ):
            # --- tiny constants first (so iteration-1 MMs aren't queued
            # behind the bulk M load on the DMA queue), then the dummy
            # warm-up AllGather which absorbs the first-collective staging
            # cost (~50us) while the M shard streams in ---
            ident_sb = consts.tile([B, B], bf16)
            nc.sync.dma_start(out=ident_sb[:], in_=ident_dram.ap())
            xt_sb = consts.tile([128, KT * B], bf16)
            nc.sync.dma_start(out=xt_sb[:], in_=xt_dram.ap())
            xsh_sb = consts.tile([B, DK], bf16)
            nc.sync.dma_start(out=xsh_sb[:], in_=xsh_dram.ap())

            warm_in = dram.tile([64], bf16, tag="warm_in")
            warm_out = dram.tile([64 * NCORES], bf16, tag="warm_out")
            nc.sync.dma_start(
                out=warm_in.rearrange("(p c) -> p c", p=8), in_=ident_sb[:]
            )
            nc.gpsimd.collective_compute(
                "AllGather", ALU.bypass, replica_groups=RG,
                ins=[warm_in[:]], outs=[warm_out[:]],
            )

            # --- resident M'' shard: 16 tiles of 4 K-tiles each so
            # iteration-1 matmuls can chase the load group by group.
            # Spread the bulk load over several engines' DMA queues. ---
            m_view = m_dram.ap().rearrange("(kt p) c -> kt p c", p=128)
            m_tiles = {}
            load_engines = [nc.sync, nc.scalar, nc.gpsimd]
            for g in range(16):
                mt = mpool.tile([128, 4 * DK], bf16, tag=f"m{g}")
                src = m_view[g * 4 : (g + 1) * 4].rearrange("kt p c -> p kt c")
                load_engines[g % len(load_engines)].dma_start(
                    out=mt.rearrange("p (kt c) -> p kt c", c=DK), in_=src
                )
                m_tiles[g] = mt

            def m_ap(ki, col0, ncol):
                """AP of M'' K-tile ki, columns [col0, col0+ncol)."""
                g, kk = divmod(ki, 4)
                return m_tiles[g][:, kk * DK + col0 : kk * DK + col0 + ncol]

            cur_vT = xt_sb  # iteration-1 stationary operand = bf16(x)^T

            for t in range(tau):
                last = t == tau - 1

                ps = [
                    mmps.tile([B, 512], fp32, tag="ps", name=f"ps{t}_{h}")
                    for h in range(2)
                ]
                nxt_vT = None if last else state.tile([128, KT * B], bf16)

                def mm_block(kis, half, start, stop):
                    for i, ki in enumerate(kis):
                        nc.tensor.matmul(
                            ps[half][:],
                            cur_vT[:, ki * B : (ki + 1) * B],
                            m_ap(ki, half * 512, 512),
                            start=(start and i == 0),
                            stop=(stop and i == len(kis) - 1),
                        )

                def half_cast(half):
                    """psum -> bf16 (iter 1: also subtract the baked decay,
                    since the reference's first step has hs=0)."""
                    q = qpool.tile([B, 512], bf16, tag="q", name=f"q{t}_{half}")
                    if t == 0:
                        nc.vector.scalar_tensor_tensor(
                            out=q[:],
                            in0=xsh_sb[:, half * 512 : half * 512 + 512],
                            scalar=-0.5 * LAM,
                            in1=ps[half][:],
                            op0=ALU.mult,
                            op1=ALU.add,
                        )
                    else:
                        nc.vector.tensor_copy(out=q[:], in_=ps[half][:])
                    return q

                def half_transpose(half, q):
                    tr = trps.tile([128, 4 * B], bf16, tag="tr",
                                   name=f"tr{t}_{half}")
                    for m in range(4):
                        nc.tensor.transpose(
                            tr[:, m * B : (m + 1) * B],
                            q[:, m * 128 : (m + 1) * 128],
                            ident_sb[:],
                        )
                    return tr

                def half_gather(half, tr):
                    """lrelu -> DMA out -> AllGather -> DMA into next state."""
                    a_T = tvec.tile([128, 4 * B], bf16, tag="aT",
                                    name=f"aT{t}_{half}")
                    nc.vector.tensor_scalar_mul(a_T[:], tr[:], SLOPE)
                    w_T = tvec.tile([128, 4 * B], bf16, tag="wT",
                                    name=f"wT{t}_{half}")
                    nc.vector.tensor_tensor(
                        out=w_T[:], in0=tr[:], in1=a_T[:], op=ALU.max
                    )
                    ag_in = dram.tile([128 * 4 * B], bf16, tag="ag_in",
                                      name=f"agi{t}_{half}")
                    ag_out = dram.tile([NCORES * 128 * 4 * B], bf16,
                                       tag="ag_out", name=f"ago{t}_{half}")
                    nc.sync.dma_start(
                        out=ag_in.rearrange("(p c) -> p c", p=128), in_=w_T[:]
                    )
                    nc.gpsimd.collective_compute(
                        "AllGather", ALU.bypass, replica_groups=RG,
                        ins=[ag_in[:]], outs=[ag_out[:]],
                    )
                    # gathered rank blocks -> interleaved state columns:
                    # rank r half h lands at vT[:, r*64 + 32h : r*64 + 32h+32]
                    # (split across two DMA queues for descriptor-rate
                    # parallelism: the pattern is 64B-granular)
                    dst = nxt_vT[:].rearrange("p (r c) -> p r c", c=8 * B)[
                        :, :, half * 4 * B : (half + 1) * 4 * B
                    ]
                    src = ag_out.rearrange("(r p c) -> p r c", p=128, c=4 * B)
                    nc.sync.dma_start(out=dst[:, 0:4], in_=src[:, 0:4])
                    nc.scalar.dma_start(out=dst[:, 4:8], in_=src[:, 4:8])

                if last:
                    # f32 leaky-relu on the shard, write output; host
                    # normalizes (scale drops out).
                    mm_block(A_KI, 0, True, False)
                    mm_block(A_KI, 1, True, False)
                    mm_block(B_KI, 0, False, True)
                    mm_block(B_KI, 1, False, True)
                    o_f = fin.tile([B, DK], fp32)
                    for half in range(2):
                        a_f = fin.tile([B, 512], fp32, tag="af")
                        nc.vector.tensor_scalar_mul(a_f[:], ps[half][:], SLOPE)
                        nc.vector.tensor_tensor(
                            out=o_f[:, half * 512 : half * 512 + 512],
                            in0=ps[half][:],
                            in1=a_f[:],
                            op=ALU.max,
                        )
                    nc.sync.dma_start(out=out_dram.ap(), in_=o_f[:])
                    continue

                if t == 0:
                    # iteration 1 chases the M load group by group (its
                    # operand xt is resident from the start)
                    GRP = [list(range(g * 4, (g + 1) * 4)) for g in range(16)]
                    for g in range(16):
                        mm_block(GRP[g], 0, g == 0, g == 15)
                    q0 = half_cast(0)
                    mm_block(GRP[0], 1, True, False)
                    tr0 = half_transpose(0, q0)
                    half_gather(0, tr0)
                    for g in range(1, 16):
                        mm_block(GRP[g], 1, False, g == 15)
                    q1 = half_cast(1)
                    tr1 = half_transpose(1, q1)
                    half_gather(1, tr1)
                else:
                    # steady state: A-tiles (gathered by AG#1 of the previous
                    # step) first, h0 completes ~60% into the stream so AG#1
                    # rides under the rest; a few h1 MMs cover the cast
                    # latency before the transposes.
                    mm_block(A_KI, 0, True, False)
                    mm_block(A_KI[:16], 1, True, False)
                    mm_block(B_KI, 0, False, True)
                    q0 = half_cast(0)
                    mm_block(A_KI[16:20], 1, False, False)
                    tr0 = half_transpose(0, q0)
                    half_gather(0, tr0)
                    mm_block(A_KI[20:], 1, False, False)
                    mm_block(B_KI, 1, False, True)
                    q1 = half_cast(1)
                    tr1 = half_transpose(1, q1)
                    half_gather(1, tr1)

                cur_vT = nxt_vT

    nc.finalize()
    return nc


def _get_program(tau=TAU):
    if tau not in _cached:
        _cached[tau] = _build_program(tau)
    return _cached[tau]


def _prep_inputs(x, M):
    """Host-side shard prep. Returns list of 8 per-core input dicts."""
    lam = np.float32(LAM)
    xt = (
        x.reshape(B, KT, 128)
        .transpose(2, 1, 0)  # [128, KT, B]
        .reshape(128, KT * B)
        .astype(_BF16)
    )
    ident = np.eye(B, dtype=np.float32).astype(_BF16)
    in_maps = []
    idx = np.arange(DK)
    for r in range(NCORES):
        cols = slice(r * DK, (r + 1) * DK)
        m_shard = M[:, cols] * lam
        m_shard[r * DK + idx, idx] += np.float32(0.5) * lam
        in_maps.append(
            {
                "m": m_shard.astype(_BF16),
                "xt": xt,
                "xsh": np.ascontiguousarray(x[:, cols]).astype(_BF16),
                "ident": ident,
            }
        )
    return in_maps


def kernel(x, M, hs):
    """Full-input entry point: shards internally across 8 NeuronCores."""
    from concourse.bass_utils import run_bass_kernel_spmd

    x = np.asarray(x, dtype=np.float32)
    M = np.asarray(M, dtype=np.float32)
    nc = _get_program()
    in_maps = _prep_inputs(x, M)
    res = run_bass_kernel_spmd(nc, in_maps, core_ids=list(range(NCORES)))
    shards = [res.results[r]["out"] for r in range(NCORES)]
    v = np.concatenate(shards, axis=1)  # [8, 8192] f32, unnormalized act_16
    # Normalize in f64 WITHOUT the reference's 1e-12 clamp: our v carries an
    # arbitrary per-row scale (collapsed all-negative rows shrink ~100x per
    # step), so the clamp must scale with it; the reference's clamp never
    # fires for its own normalized state.
    v64 = v.astype(np.float64)
    nrm = np.sqrt((v64 ** 2).sum(axis=1, keepdims=True))
    return (v64 / nrm).astype(np.float32)
